# revision 6
# baseline (speedup 1.0000x reference)
"""Trainium2 Bass kernel for multi-head attention with RoPE.

Problem: b=8, n=1024, d_model=768, heads=12, dim_head=64.
Strategy: data parallel over batch — each of the 8 NeuronCores handles one
batch element end-to-end (QKV proj + RoPE + attention + out proj). No
collectives needed.

v2: the attention phase is ACT(exp)-bound (~110us of serial exp) while PE
has ~163us of work. Emission order is per-engine FIFO, so the kernel is
scheduled so the exp chain starts early (~30us) and hides under PE work:
  - weight DMAs split per-tile (w_qkv repacked pair-major on host) so the
    first q/k projection pair unblocks at ~7us;
  - V-projection tiles, later qk pairs, and out-projection partials are
    emitted as PE filler between attention steps (shared psS slot
    rotation), keeping ACT pure-exp;
  - out projection accumulates per-e partials into SBUF f32 accumulators
    (DVE tensor_add, seeded with the bias) so only the e=5 step remains
    after the last attention pair;
  - softmax normalize: reciprocal + DRAM-broadcast, with the aT multiply
    deferred into the next pair (avoids DVE head-of-line stall on the
    broadcast DMA); the final pair instead broadcasts via a tiny ones
    matmul into PSUM to cut the DRAM round-trip from the tail.

Per-core math (all in transposed [feature, token] layout so every matmul
contraction sits on the partition axis; operands padded to the full 128
partitions for full SBUF-stream bandwidth):
  xT   [768,1024]  = x^T             (bf16, transposed on host)
  qT   [768,1024]  = Wq^T x^T        then RoPE in bf16 on DVE
  kz   2x[128,1024] per head pair: rotated k rows zero-padded to K=128
  V    [1024,12*128] = x Wv, 128 cols/head: 64 v | ones col | zeros
  per head pair (software-pipelined one step):
    sT[j,i] = sum_d kz[d,j] qT[d,i]  (K=128 contraction, zeros inert)
    pT  = exp(sT / 8)                (no max-subtraction; |S/8| <~ 6)
    oT[128,1024] += PV accum over j tiles; row 64 = softmax denominators
    aT = oT[0:64] * bcast(1/oT[64])
  out [1024,768] = sum_e aT[e]^T Wout[e] + b   (SBUF-accumulated)
"""

import os
import numpy as np
import ml_dtypes

N = 1024
D = 768
H = 12
DH = 64
E3 = 2304
KT = 6          # number of 128-row tiles of the model dim (768/128)
NT = 8          # number of 128-token tiles (1024/128)
P = 128
N_CORES = 8
VW = 65         # per-head V width incl. ones column

_CACHE = {}


def _build():
    import concourse.bass as bass
    import concourse.mybir as mybir
    import concourse.tile as tile
    from concourse import bacc

    F32 = mybir.dt.float32
    BF16 = mybir.dt.bfloat16
    Exp = mybir.ActivationFunctionType.Exp

    nc = bacc.Bacc("TRN2", target_bir_lowering=False, debug=False,
                   num_devices=N_CORES)

    x = nc.dram_tensor("x", [D, N], BF16, kind="ExternalInput")
    # pair-major qk weights: [p, hp*1536 + k*256 + u*128 + c], u=0 -> q
    wqkp = nc.dram_tensor("wqkp", [P, KT * 1536], BF16, kind="ExternalInput")
    wv_d = nc.dram_tensor("wv_d", [P, KT * D], BF16, kind="ExternalInput")
    wout = nc.dram_tensor("wout", [P, KT * D], BF16, kind="ExternalInput")
    cos2 = nc.dram_tensor("cos2", [P, N], BF16, kind="ExternalInput")
    sins2 = nc.dram_tensor("sins2", [P, N], BF16, kind="ExternalInput")
    biasb = nc.dram_tensor("biasb", [P, D], F32, kind="ExternalInput")
    out = nc.dram_tensor("out", [N, D], F32, kind="ExternalOutput")

    with tile.TileContext(nc, pool_alloc_mode="queue") as tc:
        import contextlib
        with contextlib.ExitStack() as ctx:
            persist = ctx.enter_context(tc.tile_pool(name="persist", bufs=1))
            scr = ctx.enter_context(tc.tile_pool(name="scr", bufs=4))
            ptp = ctx.enter_context(tc.tile_pool(name="ptp", bufs=4))
            smallp = ctx.enter_context(tc.tile_pool(name="smallp", bufs=1))
            otp = ctx.enter_context(tc.tile_pool(name="otp", bufs=2))
            outp = ctx.enter_context(tc.tile_pool(name="outp", bufs=2))
            dramp = ctx.enter_context(
                tc.tile_pool(name="dram", bufs=2, space="DRAM"))

            # ---- startup DMAs, ordered by first use: x + pair-0 weights
            # gate the first q/k projection; everything else streams in
            # behind them.
            xT = [persist.tile([P, N], BF16, tag=f"xT{t_i}",
                               name=f"xT_sb{t_i}") for t_i in range(KT)]
            wqk_sb = [persist.tile([P, 1536], BF16, tag=f"wqk{hp}",
                                   name=f"wqk_sb{hp}") for hp in range(6)]
            wv_sb = [persist.tile([P, D], BF16, tag=f"wv{k}",
                                  name=f"wv_sb{k}") for k in range(KT)]
            wo_sb = [persist.tile([P, D], BF16, tag=f"wo{e}",
                                  name=f"wo_sb{e}") for e in range(KT)]
            for t_i in range(KT):
                nc.sync.dma_start(xT[t_i][:], x[t_i * P:(t_i + 1) * P, :])
            nc.sync.dma_start(wqk_sb[0][:], wqkp[:, 0:1536])
            cos_sb = persist.tile([P, N], BF16, tag="cos", name="cos_sb")
            nc.sync.dma_start(cos_sb[:], cos2[:, :])
            sin_sb = persist.tile([P, N], BF16, tag="sin", name="sin_sb")
            nc.sync.dma_start(sin_sb[:], sins2[:, :])
            bias_sb = persist.tile([P, D], F32, tag="bias", name="bias_sb")
            nc.sync.dma_start(bias_sb[:], biasb[:, :])
            for k in range(KT):
                nc.sync.dma_start(wv_sb[k][:], wv_d[:, k * D:(k + 1) * D])
            nc.sync.dma_start(wqk_sb[1][:], wqkp[:, 1536:2 * 1536])
            for hp in range(2, 6):
                nc.sync.dma_start(wqk_sb[hp][:],
                                  wqkp[:, hp * 1536:(hp + 1) * 1536])
            for e in range(KT):
                nc.sync.dma_start(wo_sb[e][:], wout[:, e * D:(e + 1) * D])

            # ---- persistent SBUF state
            qkT = [persist.tile([P, N], BF16, tag=f"qkT{m}", name=f"qkT_sb{m}")
                   for m in range(6)]
            kz = [[persist.tile([P, N], BF16, tag=f"kz{hp}_{u}",
                                name=f"kz_sb{hp}_{u}") for u in range(2)]
                  for hp in range(6)]
            vt = [persist.tile([P, H * P], BF16, tag=f"vt{n}", name=f"vt_sb{n}")
                  for n in range(NT)]
            aT = [persist.tile([P, N], BF16, tag=f"aT{e}", name=f"aT_sb{e}")
                  for e in range(KT)]
            acc = [persist.tile([P, D], F32, tag=f"acc{it}",
                                name=f"acc_sb{it}") for it in range(NT)]
            ss_sb = persist.tile([33, N], F32, tag="ss", name="ss_t")
            ones_sb = persist.tile([33, DH], F32, tag="ones", name="ones_t")

            # all memsets on the otherwise-idle GpSimd engine, upfront
            # (vt ones/zeros cols are disjoint from the scatter dst)
            for hp in range(6):
                nc.gpsimd.memset(kz[hp][0][DH:P, :], 0.0)
                nc.gpsimd.memset(kz[hp][1][0:DH, :], 0.0)
            nc.gpsimd.memset(ss_sb[0:32, :], 1.0)
            nc.gpsimd.memset(ones_sb[0:1, :], 1.0)
            nc.gpsimd.memset(ones_sb[32:33, :], 1.0)
            for ni in range(NT):
                vre = vt[ni].rearrange("p (h j) -> p h j", j=P)
                nc.gpsimd.memset(vre[:, :, DH:DH + 1], 1.0)
                nc.gpsimd.memset(vre[:, :, DH + 1:P], 0.0)

            with (tc.tile_pool(name="psS", bufs=2, space="PSUM") as psum,
                  tc.tile_pool(name="psO", bufs=2, space="PSUM") as psumO):

                # ---------- building blocks ----------
                def emit_qk_tile(hp, u, pool, tag):
                    """project one q (u=0) or k (u=1) tile of pair hp and
                    rope it into qkT[hp] / kz[hp]."""
                    ps = pool.tile([P, N], F32, tag=tag, name="ps_qk")
                    for ih in range(2):
                        for k in range(KT):
                            nc.tensor.matmul(
                                ps[:, ih * 512:(ih + 1) * 512],
                                lhsT=wqk_sb[hp][:, k * 256 + u * P:
                                                k * 256 + (u + 1) * P],
                                rhs=xT[k][:, ih * 512:(ih + 1) * 512],
                                start=(k == 0), stop=(k == KT - 1))
                    # RoPE in bf16 (DVE); psum drain copy on ACT
                    qf = scr.tile([P, N], BF16, tag="qf", name="qf_t")
                    nc.scalar.copy(qf[:], ps[:])
                    qa = scr.tile([P, N], BF16, tag="qa", name="qa_t")
                    nc.vector.tensor_mul(qa[:], qf[:], cos_sb[:])
                    qb = scr.tile([P, N], BF16, tag="qb", name="qb_t")
                    # sin table pre-swapped on host so in0/in1 share a base
                    # partition; only the output is quadrant-shifted.
                    for blk in range(4):
                        ob = blk * 32
                        ib = (blk ^ 1) * 32  # 0<->32, 64<->96
                        nc.vector.tensor_mul(
                            qb[ob:ob + 32, :], qf[ib:ib + 32, :],
                            sin_sb[ib:ib + 32, :])
                    if u == 0:
                        nc.vector.tensor_add(qkT[hp][:], qa[:], qb[:])
                    else:
                        nc.vector.tensor_add(
                            kz[hp][0][0:DH, :], qa[0:DH, :], qb[0:DH, :])
                        nc.vector.tensor_add(
                            kz[hp][1][DH:P, :], qa[DH:P, :], qb[DH:P, :])

                def emit_v_tile(ni, pool, tag, use_act):
                    """project V for token tile ni into vt[ni] (64 cols per
                    head, ones/zeros already memset)."""
                    ps = pool.tile([P, N], F32, tag=tag, name="ps_v")
                    for (c0, cw) in ((0, 512), (512, 256)):
                        for k in range(KT):
                            nc.tensor.matmul(
                                ps[:, c0:c0 + cw],
                                lhsT=xT[k][:, ni * P:(ni + 1) * P],
                                rhs=wv_sb[k][:, c0:c0 + cw],
                                start=(k == 0), stop=(k == KT - 1))
                    cp = nc.scalar.copy if use_act else nc.vector.tensor_copy
                    dst8 = vt[ni][:, 0:8 * P].rearrange(
                        "p (h j) -> p h j", j=P)[:, :, 0:DH]
                    src8 = ps[:, 0:512].rearrange("p (h j) -> p h j", j=DH)
                    cp(dst8, src8)
                    dst4 = vt[ni][:, 8 * P:12 * P].rearrange(
                        "p (h j) -> p h j", j=P)[:, :, 0:DH]
                    src4 = ps[:, 512:768].rearrange("p (h j) -> p h j", j=DH)
                    cp(dst4, src4)

                def emit_out_partial(e, it):
                    """one out-proj partial: acc[it] += aT[e] tile @ wo[e]"""
                    f_ps = psum.tile([P, N], F32, tag="ps", name="f_ps")
                    for (c0, cw) in ((0, 512), (512, 256)):
                        nc.tensor.matmul(
                            f_ps[:, c0:c0 + cw],
                            lhsT=aT[e][:, it * P:(it + 1) * P],
                            rhs=wo_sb[e][:, c0:c0 + cw],
                            start=True, stop=True)
                    nc.vector.tensor_add(acc[it][:], acc[it][:],
                                         f_ps[:, 0:D])

                # ---------- attention ----------
                o_ps_all = [[None, None] for _ in range(6)]
                oTc_all = [None] * 6
                rb_all = [None] * 6

                def emit_s_exp(hp, j):
                    qt = qkT[hp]
                    s_ps = [psum.tile([P, N], F32, tag="ps",
                                      name=f"s_ps{u}") for u in range(2)]
                    for ih in range(2):
                        for u in range(2):
                            nc.tensor.matmul(
                                s_ps[u][:, ih * 512:(ih + 1) * 512],
                                lhsT=kz[hp][u][:, j * P:(j + 1) * P],
                                rhs=qt[:, ih * 512:(ih + 1) * 512],
                                start=True, stop=True)
                    pT = [None, None]
                    for u in range(2):
                        pT[u] = ptp.tile([P, N], BF16, tag="pT",
                                         name=f"pT_t{u}")
                        nc.scalar.activation(pT[u][:], s_ps[u][:], Exp,
                                             scale=0.125)
                    return pT

                def emit_pv(hp, j, pT):
                    o_ps = o_ps_all[hp]
                    if j == 0:
                        o_ps[0] = psumO.tile([P, N], F32, tag="ops",
                                             name="o_ps0")
                        o_ps[1] = psumO.tile([P, N], F32, tag="ops",
                                             name="o_ps1")
                    for u in range(2):
                        h = 2 * hp + u
                        for ih in range(2):
                            nc.tensor.matmul(
                                o_ps[u][:, ih * 512:(ih + 1) * 512],
                                lhsT=vt[j][:, h * P:(h + 1) * P],
                                rhs=pT[u][:, ih * 512:(ih + 1) * 512],
                                start=(j == 0), stop=(j == NT - 1))
                    if j == NT - 1:
                        emit_drain(hp, o_ps)

                def emit_drain(hp, o_ps):
                    """drain o_ps fast (copies + reciprocal + broadcast
                    start); the aT multiply is deferred (emit_finish)."""
                    oTc = otp.tile([P, N], BF16, tag="oT", name="oT_t")
                    oTc_all[hp] = oTc
                    if hp == 5:
                        # last pair: reciprocal chain gates the out proj —
                        # run it first and broadcast via a ones-matmul in
                        # PSUM instead of the DRAM round trip.
                        nc.vector.tensor_copy(ss_sb[0:1, :],
                                              o_ps[0][DH:DH + 1, :])
                        nc.vector.tensor_copy(ss_sb[32:33, :],
                                              o_ps[1][DH:DH + 1, :])
                        r_sb = smallp.tile([33, N], F32, tag="r",
                                           name="r_t")
                        nc.vector.reciprocal_approx_fast(r_sb[:], ss_sb[:])
                        rb_ps = psum.tile([P, N], F32, tag="ps",
                                          name="rb_ps")
                        for ih in range(2):
                            nc.tensor.matmul(
                                rb_ps[0:DH, ih * 512:(ih + 1) * 512],
                                lhsT=ones_sb[0:1, :],
                                rhs=r_sb[0:1, ih * 512:(ih + 1) * 512],
                                start=True, stop=True)
                            nc.tensor.matmul(
                                rb_ps[DH:P, ih * 512:(ih + 1) * 512],
                                lhsT=ones_sb[32:33, :],
                                rhs=r_sb[32:33, ih * 512:(ih + 1) * 512],
                                start=True, stop=True,
                                skip_group_check=True)
                        nc.scalar.copy(oTc[0:DH, :], o_ps[0][0:DH, :])
                        nc.vector.tensor_copy(oTc[DH:P, :],
                                              o_ps[1][0:DH, :])
                        nc.vector.tensor_mul(aT[5][:], oTc[:], rb_ps[:])
                    else:
                        nc.vector.tensor_copy(oTc[0:DH, :],
                                              o_ps[0][0:DH, :])
                        nc.vector.tensor_copy(oTc[DH:P, :],
                                              o_ps[1][0:DH, :])
                        nc.vector.tensor_copy(ss_sb[0:1, :],
                                              o_ps[0][DH:DH + 1, :])
                        nc.vector.tensor_copy(ss_sb[32:33, :],
                                              o_ps[1][DH:DH + 1, :])
                        r_sb = smallp.tile([33, N], F32, tag="r",
                                           name="r_t")
                        nc.vector.reciprocal_approx_fast(r_sb[:], ss_sb[:])
                        r_dr = dramp.tile([2, N], F32, tag="rdr",
                                          name="rdr_t")
                        nc.sync.dma_start(r_dr[0:1, :], r_sb[0:1, :])
                        nc.sync.dma_start(r_dr[1:2, :], r_sb[32:33, :])
                        rb_sb = smallp.tile([P, N], F32, tag="rb",
                                            name="rb_t")
                        nc.sync.dma_start(rb_sb[0:DH, :],
                                          r_dr[0:1, :].broadcast_to([DH, N]))
                        nc.sync.dma_start(rb_sb[DH:P, :],
                                          r_dr[1:2, :].broadcast_to([DH, N]))
                        rb_all[hp] = rb_sb

                def emit_finish(hp):
                    # deferred aT multiply: by now the broadcast DMA landed
                    nc.vector.tensor_mul(aT[hp][:], oTc_all[hp][:],
                                         rb_all[hp][:])

                # ---------- emission schedule ----------
                # pre-attention: pair 0+1 projections and V tiles 0-3
                emit_qk_tile(0, 0, psum, "ps")
                emit_qk_tile(0, 1, psumO, "ops")
                emit_v_tile(0, psum, "ps", use_act=True)
                emit_v_tile(1, psumO, "ops", use_act=True)
                emit_qk_tile(1, 0, psum, "ps")
                emit_qk_tile(1, 1, psumO, "ops")
                # seed the out-proj accumulators with the bias (emitted
                # here so the DVE FIFO doesn't stall pair-0 rope on the
                # bias DMA; first use is at attention step (2,2))
                for it in range(NT):
                    nc.vector.tensor_copy(acc[it][:], bias_sb[:])

                # filler closures keyed by (hp, j): emitted after that
                # attention step's PV in program order.
                def V(ni):
                    return lambda: emit_v_tile(ni, psum, "ps", use_act=False)

                def QK(hp, u):
                    return lambda: emit_qk_tile(hp, u, psum, "ps")

                def OP(e, it):
                    return lambda: emit_out_partial(e, it)

                def FIN(hp):
                    return lambda: emit_finish(hp)

                fillers = {
                    (0, 0): [V(2)], (0, 1): [V(3)], (0, 2): [V(4)],
                    (0, 3): [V(5)], (0, 4): [V(6)], (0, 5): [V(7)],
                    (1, 1): [QK(2, 0)], (1, 3): [FIN(0)],
                    (1, 4): [QK(2, 1)],
                    (2, 1): [QK(3, 0)], (2, 2): [OP(0, 0), OP(0, 1)],
                    (2, 3): [FIN(1)], (2, 4): [QK(3, 1)],
                    (2, 5): [OP(0, 2), OP(0, 3)],
                    (3, 1): [QK(4, 0)], (3, 2): [OP(0, 4), OP(0, 5)],
                    (3, 3): [FIN(2)], (3, 4): [QK(4, 1)],
                    (3, 5): [OP(0, 6), OP(0, 7)],
                    (3, 6): [OP(1, 0), OP(1, 1)],
                    (4, 1): [QK(5, 0)], (4, 2): [OP(1, 2), OP(1, 3)],
                    (4, 3): [FIN(3)], (4, 4): [QK(5, 1)],
                    (4, 5): [OP(1, 4), OP(1, 5)],
                    (4, 6): [OP(1, 6), OP(1, 7)],
                    (4, 7): [OP(2, 0), OP(2, 1)],
                    (5, 0): [OP(2, 2), OP(2, 3)],
                    (5, 1): [OP(2, 4), OP(2, 5)],
                    (5, 2): [OP(2, 6), OP(2, 7)],
                    (5, 3): [FIN(4), OP(3, 0), OP(3, 1)],
                    (5, 4): [OP(3, 2), OP(3, 3), OP(4, 0)],
                    (5, 5): [OP(3, 4), OP(3, 5), OP(4, 1)],
                    (5, 6): [OP(3, 6), OP(3, 7), OP(4, 2)],
                    (5, 7): [OP(4, 3), OP(4, 4)],
                }

                steps = [(hp, j) for hp in range(6) for j in range(NT)]
                prev = None
                for st in steps:
                    pT = emit_s_exp(*st)
                    if prev is not None:
                        emit_pv(*prev)
                        for f in fillers.get(prev[:2], []):
                            f()
                    prev = (st[0], st[1], pT)
                emit_pv(*prev)
                for f in fillers.get(prev[:2], []):
                    f()
                # leftover e=4 partials (aT[4] ready since (5,3))
                for it in range(5, NT):
                    emit_out_partial(4, it)

                # ---- terminal: e=5 partial + combine + store ----
                for it in range(NT):
                    f_pool = psumO if it % 2 == 0 else psum
                    f_ps = f_pool.tile([P, N], F32,
                                       tag="ops" if it % 2 == 0 else "ps",
                                       name="f5_ps")
                    for (c0, cw) in ((0, 512), (512, 256)):
                        nc.tensor.matmul(
                            f_ps[:, c0:c0 + cw],
                            lhsT=aT[5][:, it * P:(it + 1) * P],
                            rhs=wo_sb[5][:, c0:c0 + cw],
                            start=True, stop=True)
                    o_sb = outp.tile([P, D], F32, tag="osb", name="osb_t")
                    nc.vector.tensor_add(o_sb[:], f_ps[:, 0:D], acc[it][:])
                    nc.sync.dma_start(out[it * P:(it + 1) * P, :], o_sb[:])

    nc.compile()
    return nc


def _host_tables():
    inv_freq = 1.0 / (10000.0 ** (np.arange(0, DH, 2, dtype=np.float32) / DH))
    t = np.arange(N, dtype=np.float32)
    freqs = np.einsum("i,j->ij", t, inv_freq)          # [N, 32]
    emb = np.concatenate([freqs, freqs], axis=-1)      # [N, 64]
    cosT = np.cos(emb).T.astype(np.float32)            # [64, N]
    sinT = np.sin(emb).T.astype(np.float32)            # [64, N]
    # b-term: out rows 0:32 use -sin (pair d+32), rows 32:64 use +sin
    sins = np.concatenate([-sinT[0:32], sinT[32:64]], axis=0)  # [64, N]
    cos2 = np.concatenate([cosT, cosT], axis=0)        # [128, N]
    sins2 = np.concatenate([sins, sins], axis=0)       # [128, N]
    # pre-swap 32-row blocks (0<->32, 64<->96): the device multiplies
    # qb[ob] = qf[ib] * sin_sb[ib], so sin_sb[ib] must hold sins2[ob].
    sinsw2 = np.concatenate(
        [sins2[32:64], sins2[0:32], sins2[96:128], sins2[64:96]], axis=0)
    return np.ascontiguousarray(cos2), np.ascontiguousarray(sinsw2)


def kernel(x, w_qkv, w_out, b_out):
    from concourse.bass_utils import run_bass_kernel_spmd

    if "nc" not in _CACHE:
        _CACHE["nc"] = _build()
    nc = _CACHE["nc"]

    bf = ml_dtypes.bfloat16
    cos2, sins2 = _host_tables()
    cos2 = np.ascontiguousarray(cos2.astype(bf))
    sins2 = np.ascontiguousarray(sins2.astype(bf))
    biasb = np.ascontiguousarray(
        np.broadcast_to(np.asarray(b_out, np.float32)[None, :], (P, D)))

    def _sbufize(w):   # [(k p), e] -> [p, (k e)] exact SBUF layout
        w = np.asarray(w, np.float32).astype(bf)
        k, e = w.shape[0] // P, w.shape[1]
        return np.ascontiguousarray(
            w.reshape(k, P, e).transpose(1, 0, 2).reshape(P, k * e))

    # pair-major qk weights: [p, hp*1536 + k*256 + u*128 + c], m = hp + 6u
    w1 = np.asarray(w_qkv, np.float32)[:, 0:1536].astype(bf)
    wqkp = np.ascontiguousarray(
        w1.reshape(KT, P, 2, 6, P).transpose(1, 3, 0, 2, 4)
        .reshape(P, KT * 1536))
    wv_b = _sbufize(np.asarray(w_qkv, np.float32)[:, 1536:E3])
    wout_b = _sbufize(w_out)

    in_maps = []
    for i in range(N_CORES):
        xi = np.ascontiguousarray(
            np.asarray(x[i], np.float32).astype(bf).T)
        in_maps.append({
            "x": xi, "wqkp": wqkp, "wv_d": wv_b, "wout": wout_b,
            "cos2": cos2, "sins2": sins2, "biasb": biasb,
        })

    res = run_bass_kernel_spmd(
        nc, in_maps, list(range(N_CORES)),
        trace=bool(int(os.environ.get("KERNEL_TRACE", "0"))))
    _CACHE["last_result"] = res
    return np.stack([res.results[i]["out"] for i in range(N_CORES)], axis=0)


# revision 7
# speedup vs baseline: 1.2606x; 1.2606x over previous
"""Trainium2 Bass kernel for multi-head attention with RoPE.

Problem: b=8, n=1024, d_model=768, heads=12, dim_head=64.
Strategy: data parallel over batch — each of the 8 NeuronCores handles one
batch element end-to-end (QKV proj + RoPE + attention + out proj). No
collectives needed.

v3: the attention phase is ACT(exp)-bound (~110us of serial exp) while PE
has ~163us of work; PSUM (8 banks) is fully booked by attention (4 banks
S double-buffer + 4 banks PV accumulators), so any concurrent GEMM must
share the S-tile slot rotation. Engine queues execute in emission order,
so the schedule is:
  - DMA wave 1 interleaves xT row-tiles with per-k chunks of the pair-0
    qk weights (w_qkv repacked pair-major on host), so the first
    projection starts ~10us in and the exp chain starts ~20us in;
  - attention pair p hosts PE filler: V tiles (pair 0), the qk projection
    of pair p+1 (pairs 1-4), and the out-projection (pair 5);
  - out projection: e=0..4 accumulate in one PSUM tile per token tile
    during pair 5 (aT[0..4] all exist by then), drained once to SBUF f32
    accumulators with the bias folded in; only e=5 + combine remain after
    the last pair — and its reciprocal broadcast avoids the DRAM round
    trip via a tiny ones-matmul into PSUM;
  - softmax normalize for pairs 0-4: reciprocal + DRAM-broadcast with the
    aT multiply deferred into the next pair (no DVE head-of-line stall);
    drains free the PV accumulator banks one at a time;
  - ACT runs only: exps + qf drains (pairs 0-1) + V scatters (pair-0
    slack); DVE runs rope, drains, deferred normalizes, and the final
    accumulator adds.

Per-core math (all in transposed [feature, token] layout so every matmul
contraction sits on the partition axis; operands padded to the full 128
partitions for full SBUF-stream bandwidth):
  xT   [768,1024]  = x^T             (bf16, transposed on host)
  qT   [768,1024]  = Wq^T x^T        then RoPE in bf16 on DVE
  kz   2x[128,1024] per head pair: rotated k rows zero-padded to K=128
  V    [1024,12*128] = x Wv, 128 cols/head: 64 v | ones col | zeros
  per head pair (software-pipelined one step):
    sT[j,i] = sum_d kz[d,j] qT[d,i]  (K=128 contraction, zeros inert)
    pT  = exp(sT / 8)                (no max-subtraction; |S/8| <~ 6)
    oT[128,1024] += PV accum over j tiles; row 64 = softmax denominators
    aT = oT[0:64] * bcast(1/oT[64])
  out [1024,768] = sum_e aT[e]^T Wout[e] + b   (PSUM e<5, SBUF + e=5)
"""

import os
import numpy as np
import ml_dtypes

N = 1024
D = 768
H = 12
DH = 64
E3 = 2304
KT = 6          # number of 128-row tiles of the model dim (768/128)
NT = 8          # number of 128-token tiles (1024/128)
P = 128
N_CORES = 8

_CACHE = {}


def _build():
    import concourse.bass as bass
    import concourse.mybir as mybir
    import concourse.tile as tile
    from concourse import bacc

    F32 = mybir.dt.float32
    BF16 = mybir.dt.bfloat16
    Exp = mybir.ActivationFunctionType.Exp
    Alu = mybir.AluOpType

    nc = bacc.Bacc("TRN2", target_bir_lowering=False, debug=False,
                   num_devices=N_CORES)

    x = nc.dram_tensor("x", [D, N], BF16, kind="ExternalInput")
    # pair-major qk weights: [p, hp*1536 + k*256 + u*128 + c], u=0 -> q
    wqkp = nc.dram_tensor("wqkp", [P, KT * 1536], BF16, kind="ExternalInput")
    wv_d = nc.dram_tensor("wv_d", [P, KT * D], BF16, kind="ExternalInput")
    wout = nc.dram_tensor("wout", [P, KT * D], BF16, kind="ExternalInput")
    cos2 = nc.dram_tensor("cos2", [P, N], BF16, kind="ExternalInput")
    sins2 = nc.dram_tensor("sins2", [P, N], BF16, kind="ExternalInput")
    biasb = nc.dram_tensor("biasb", [P, D], F32, kind="ExternalInput")
    out = nc.dram_tensor("out", [N, D], F32, kind="ExternalOutput")

    with tile.TileContext(nc, pool_alloc_mode="queue") as tc:
        import contextlib
        with contextlib.ExitStack() as ctx:
            persist = ctx.enter_context(tc.tile_pool(name="persist", bufs=1))
            scr = ctx.enter_context(tc.tile_pool(name="scr", bufs=4))
            ptp = ctx.enter_context(tc.tile_pool(name="ptp", bufs=4))
            smallp = ctx.enter_context(tc.tile_pool(name="smallp", bufs=1))
            otp = ctx.enter_context(tc.tile_pool(name="otp", bufs=2))
            outp = ctx.enter_context(tc.tile_pool(name="outp", bufs=2))
            dramp = ctx.enter_context(
                tc.tile_pool(name="dram", bufs=2, space="DRAM"))

            # ---- startup DMAs; DMA service is ~in emission order, so
            # wave 1 is exactly what the pair-0 qk projection consumes,
            # k-interleaved so the k-accumulation chases arrivals.
            xT = [persist.tile([P, N], BF16, tag=f"xT{t_i}",
                               name=f"xT_sb{t_i}") for t_i in range(KT)]
            wqk_sb = [persist.tile([P, 1536], BF16, tag=f"wqk{hp}",
                                   name=f"wqk_sb{hp}") for hp in range(6)]
            wv_sb = [persist.tile([P, D], BF16, tag=f"wv{k}",
                                  name=f"wv_sb{k}") for k in range(KT)]
            wo_sb = [persist.tile([P, D], BF16, tag=f"wo{e}",
                                  name=f"wo_sb{e}") for e in range(KT)]
            for t_i in range(KT):
                nc.sync.dma_start(xT[t_i][:], x[t_i * P:(t_i + 1) * P, :])
                nc.sync.dma_start(wqk_sb[0][:, t_i * 256:(t_i + 1) * 256],
                                  wqkp[:, t_i * 256:(t_i + 1) * 256])
            cos_sb = persist.tile([P, N], BF16, tag="cos", name="cos_sb")
            nc.sync.dma_start(cos_sb[:], cos2[:, :])
            sin_sb = persist.tile([P, N], BF16, tag="sin", name="sin_sb")
            nc.sync.dma_start(sin_sb[:], sins2[:, :])
            for k in range(KT):
                nc.sync.dma_start(wv_sb[k][:], wv_d[:, k * D:(k + 1) * D])
            for hp in range(1, 6):
                nc.sync.dma_start(wqk_sb[hp][:],
                                  wqkp[:, hp * 1536:(hp + 1) * 1536])
            for e in range(KT):
                nc.sync.dma_start(wo_sb[e][:], wout[:, e * D:(e + 1) * D])
            bias_sb = persist.tile([P, D], F32, tag="bias", name="bias_sb")
            nc.sync.dma_start(bias_sb[:], biasb[:, :])

            # ---- persistent SBUF state
            qkT = [persist.tile([P, N], BF16, tag=f"qkT{m}", name=f"qkT_sb{m}")
                   for m in range(6)]
            kz = [[persist.tile([P, N], BF16, tag=f"kz{hp}_{u}",
                                name=f"kz_sb{hp}_{u}") for u in range(2)]
                  for hp in range(6)]
            vt = [persist.tile([P, H * P], BF16, tag=f"vt{n}", name=f"vt_sb{n}")
                  for n in range(NT)]
            aT = [persist.tile([P, N], BF16, tag=f"aT{e}", name=f"aT_sb{e}")
                  for e in range(KT)]
            acc = [persist.tile([P, D], F32, tag=f"acc{it}",
                                name=f"acc_sb{it}") for it in range(NT)]
            ss_sb = persist.tile([33, N], F32, tag="ss", name="ss_t")
            ones_sb = persist.tile([33, DH], F32, tag="ones", name="ones_t")

            # all memsets on the otherwise-idle GpSimd engine, upfront
            # (vt ones/zeros cols are disjoint from the scatter dst)
            for hp in range(6):
                nc.gpsimd.memset(kz[hp][0][DH:P, :], 0.0)
                nc.gpsimd.memset(kz[hp][1][0:DH, :], 0.0)
            nc.gpsimd.memset(ss_sb[0:32, :], 1.0)
            nc.gpsimd.memset(ones_sb[0:1, :], 1.0)
            nc.gpsimd.memset(ones_sb[32:33, :], 1.0)
            for ni in range(NT):
                vre = vt[ni].rearrange("p (h j) -> p h j", j=P)
                nc.gpsimd.memset(vre[:, :, DH:DH + 1], 1.0)
                nc.gpsimd.memset(vre[:, :, DH + 1:P], 0.0)

            with (tc.tile_pool(name="psS", bufs=2, space="PSUM") as psum,
                  tc.tile_pool(name="psO", bufs=2, space="PSUM") as psumO):

                # ---------- building blocks ----------
                def emit_qk_tile(hp, u, pool, tag, qf_act):
                    """project one q (u=0) or k (u=1) tile of pair hp and
                    rope it into qkT[hp] / kz[hp]."""
                    ps = pool.tile([P, N], F32, tag=tag, name="ps_qk")
                    for ih in range(2):
                        for k in range(KT):
                            nc.tensor.matmul(
                                ps[:, ih * 512:(ih + 1) * 512],
                                lhsT=wqk_sb[hp][:, k * 256 + u * P:
                                                k * 256 + (u + 1) * P],
                                rhs=xT[k][:, ih * 512:(ih + 1) * 512],
                                start=(k == 0), stop=(k == KT - 1))
                    # psum drain on ACT pre-attention, DVE during it
                    qf = scr.tile([P, N], BF16, tag="qf", name="qf_t")
                    if qf_act:
                        nc.scalar.copy(qf[:], ps[:])
                    else:
                        nc.vector.tensor_copy(qf[:], ps[:])
                    qa = scr.tile([P, N], BF16, tag="qa", name="qa_t")
                    nc.vector.tensor_mul(qa[:], qf[:], cos_sb[:])
                    qb = scr.tile([P, N], BF16, tag="qb", name="qb_t")
                    # sin table pre-swapped on host so in0/in1 share a base
                    # partition; only the output is quadrant-shifted.
                    for blk in range(4):
                        ob = blk * 32
                        ib = (blk ^ 1) * 32  # 0<->32, 64<->96
                        nc.vector.tensor_mul(
                            qb[ob:ob + 32, :], qf[ib:ib + 32, :],
                            sin_sb[ib:ib + 32, :])
                    if u == 0:
                        nc.vector.tensor_add(qkT[hp][:], qa[:], qb[:])
                    else:
                        nc.vector.tensor_add(
                            kz[hp][0][0:DH, :], qa[0:DH, :], qb[0:DH, :])
                        nc.vector.tensor_add(
                            kz[hp][1][DH:P, :], qa[DH:P, :], qb[DH:P, :])

                def emit_v_tile(ni, pool, tag):
                    """project V for token tile ni into vt[ni] (64 cols per
                    head, ones/zeros already memset); scatter on ACT."""
                    ps = pool.tile([P, N], F32, tag=tag, name="ps_v")
                    for (c0, cw) in ((0, 512), (512, 256)):
                        for k in range(KT):
                            nc.tensor.matmul(
                                ps[:, c0:c0 + cw],
                                lhsT=xT[k][:, ni * P:(ni + 1) * P],
                                rhs=wv_sb[k][:, c0:c0 + cw],
                                start=(k == 0), stop=(k == KT - 1))
                    dst8 = vt[ni][:, 0:8 * P].rearrange(
                        "p (h j) -> p h j", j=P)[:, :, 0:DH]
                    src8 = ps[:, 0:512].rearrange("p (h j) -> p h j", j=DH)
                    nc.scalar.copy(dst8, src8)
                    dst4 = vt[ni][:, 8 * P:12 * P].rearrange(
                        "p (h j) -> p h j", j=P)[:, :, 0:DH]
                    src4 = ps[:, 512:768].rearrange("p (h j) -> p h j", j=DH)
                    nc.scalar.copy(dst4, src4)

                def emit_out_batch(it):
                    """out-proj partials e=0..4 for token tile it, PSUM
                    accumulated; drain once into acc[it] (+bias)."""
                    f_ps = psum.tile([P, N], F32, tag="ps", name="f_ps")
                    for (c0, cw) in ((0, 512), (512, 256)):
                        for e in range(5):
                            nc.tensor.matmul(
                                f_ps[:, c0:c0 + cw],
                                lhsT=aT[e][:, it * P:(it + 1) * P],
                                rhs=wo_sb[e][:, c0:c0 + cw],
                                start=(e == 0), stop=(e == 4))
                    nc.vector.scalar_tensor_tensor(
                        acc[it][:], f_ps[:, 0:D], 1.0, bias_sb[:],
                        Alu.mult, Alu.add)

                # ---------- attention ----------
                o_ps_all = [[None, None] for _ in range(6)]
                oTc_all = [None] * 6
                rb_all = [None] * 6

                def emit_s_exp(hp, j):
                    qt = qkT[hp]
                    s_ps = [psum.tile([P, N], F32, tag="ps",
                                      name=f"s_ps{u}") for u in range(2)]
                    for ih in range(2):
                        for u in range(2):
                            nc.tensor.matmul(
                                s_ps[u][:, ih * 512:(ih + 1) * 512],
                                lhsT=kz[hp][u][:, j * P:(j + 1) * P],
                                rhs=qt[:, ih * 512:(ih + 1) * 512],
                                start=True, stop=True)
                    pT = [None, None]
                    for u in range(2):
                        pT[u] = ptp.tile([P, N], BF16, tag="pT",
                                         name=f"pT_t{u}")
                        nc.scalar.activation(pT[u][:], s_ps[u][:], Exp,
                                             scale=0.125)
                    return pT

                def emit_pv(hp, j, pT):
                    o_ps = o_ps_all[hp]
                    if j == 0:
                        o_ps[0] = psumO.tile([P, N], F32, tag="ops",
                                             name="o_ps0")
                        o_ps[1] = psumO.tile([P, N], F32, tag="ops",
                                             name="o_ps1")
                    for u in range(2):
                        h = 2 * hp + u
                        for ih in range(2):
                            nc.tensor.matmul(
                                o_ps[u][:, ih * 512:(ih + 1) * 512],
                                lhsT=vt[j][:, h * P:(h + 1) * P],
                                rhs=pT[u][:, ih * 512:(ih + 1) * 512],
                                start=(j == 0), stop=(j == NT - 1))
                    if j == NT - 1:
                        emit_drain(hp, o_ps)

                def emit_drain(hp, o_ps):
                    """drain o_ps (frees the PV banks one at a time) +
                    reciprocal + broadcast start; the aT multiply is
                    deferred to emit_finish in the next pair."""
                    oTc = otp.tile([P, N], BF16, tag="oT", name="oT_t")
                    oTc_all[hp] = oTc
                    if hp == 5:
                        # last pair: reciprocal chain gates the out proj —
                        # run it first and broadcast via a ones-matmul in
                        # PSUM instead of the DRAM round trip.
                        nc.vector.tensor_copy(ss_sb[0:1, :],
                                              o_ps[0][DH:DH + 1, :])
                        nc.vector.tensor_copy(ss_sb[32:33, :],
                                              o_ps[1][DH:DH + 1, :])
                        r_sb = smallp.tile([33, N], F32, tag="r",
                                           name="r_t")
                        nc.vector.reciprocal_approx_fast(r_sb[:], ss_sb[:])
                        rb_ps = psum.tile([P, N], F32, tag="ps",
                                          name="rb_ps")
                        for ih in range(2):
                            nc.tensor.matmul(
                                rb_ps[0:DH, ih * 512:(ih + 1) * 512],
                                lhsT=ones_sb[0:1, :],
                                rhs=r_sb[0:1, ih * 512:(ih + 1) * 512],
                                start=True, stop=True)
                            nc.tensor.matmul(
                                rb_ps[DH:P, ih * 512:(ih + 1) * 512],
                                lhsT=ones_sb[32:33, :],
                                rhs=r_sb[32:33, ih * 512:(ih + 1) * 512],
                                start=True, stop=True,
                                skip_group_check=True)
                        nc.scalar.copy(oTc[0:DH, :], o_ps[0][0:DH, :])
                        nc.vector.tensor_copy(oTc[DH:P, :],
                                              o_ps[1][0:DH, :])
                        nc.vector.tensor_mul(aT[5][:], oTc[:], rb_ps[:])
                    else:
                        # free o_ps[0] first so PV(hp+1,0) u=0 can alloc
                        nc.vector.tensor_copy(oTc[0:DH, :],
                                              o_ps[0][0:DH, :])
                        nc.vector.tensor_copy(ss_sb[0:1, :],
                                              o_ps[0][DH:DH + 1, :])
                        nc.vector.tensor_copy(oTc[DH:P, :],
                                              o_ps[1][0:DH, :])
                        nc.vector.tensor_copy(ss_sb[32:33, :],
                                              o_ps[1][DH:DH + 1, :])
                        r_sb = smallp.tile([33, N], F32, tag="r",
                                           name="r_t")
                        nc.vector.reciprocal_approx_fast(r_sb[:], ss_sb[:])
                        r_dr = dramp.tile([2, N], F32, tag="rdr",
                                          name="rdr_t")
                        nc.sync.dma_start(r_dr[0:1, :], r_sb[0:1, :])
                        nc.sync.dma_start(r_dr[1:2, :], r_sb[32:33, :])
                        rb_sb = smallp.tile([P, N], F32, tag="rb",
                                            name="rb_t")
                        nc.sync.dma_start(rb_sb[0:DH, :],
                                          r_dr[0:1, :].broadcast_to([DH, N]))
                        nc.sync.dma_start(rb_sb[DH:P, :],
                                          r_dr[1:2, :].broadcast_to([DH, N]))
                        rb_all[hp] = rb_sb

                def emit_finish(hp):
                    # deferred aT multiply: by now the broadcast DMA landed
                    nc.vector.tensor_mul(aT[hp][:], oTc_all[hp][:],
                                         rb_all[hp][:])

                # ---------- emission schedule ----------
                # pre-attention: pair 0+1 projections and V tiles 0+1
                emit_qk_tile(0, 0, psum, "ps", qf_act=True)
                emit_qk_tile(0, 1, psumO, "ops", qf_act=True)
                emit_v_tile(0, psum, "ps")
                emit_v_tile(1, psumO, "ops")
                emit_qk_tile(1, 0, psum, "ps", qf_act=True)
                emit_qk_tile(1, 1, psumO, "ops", qf_act=True)

                def V(ni):
                    return lambda: emit_v_tile(ni, psum, "ps")

                def QK(hp, u):
                    return lambda: emit_qk_tile(hp, u, psum, "ps",
                                                qf_act=False)

                def OB(it):
                    return lambda: emit_out_batch(it)

                def FIN(hp):
                    return lambda: emit_finish(hp)

                fillers = {
                    (0, 0): [V(2)], (0, 1): [V(3)], (0, 2): [V(4)],
                    (0, 3): [V(5)], (0, 4): [V(6)], (0, 5): [V(7)],
                    (1, 1): [QK(2, 0)], (1, 3): [FIN(0)],
                    (1, 4): [QK(2, 1)],
                    (2, 1): [QK(3, 0)], (2, 3): [FIN(1)],
                    (2, 4): [QK(3, 1)],
                    (3, 1): [QK(4, 0)], (3, 3): [FIN(2)],
                    (3, 4): [QK(4, 1)],
                    (4, 1): [QK(5, 0)], (4, 3): [FIN(3)],
                    (4, 4): [QK(5, 1)],
                    (5, 1): [FIN(4)],
                    (5, 2): [OB(0)], (5, 3): [OB(1)], (5, 4): [OB(2)],
                    (5, 5): [OB(3)], (5, 6): [OB(4)], (5, 7): [OB(5)],
                }

                steps = [(hp, j) for hp in range(6) for j in range(NT)]
                prev = None
                for st in steps:
                    pT = emit_s_exp(*st)
                    if prev is not None:
                        emit_pv(*prev)
                        for f in fillers.get(prev[:2], []):
                            f()
                    prev = (st[0], st[1], pT)
                emit_pv(*prev)
                for f in fillers.get(prev[:2], []):
                    f()
                emit_out_batch(6)
                emit_out_batch(7)

                # ---- terminal: e=5 partial + combine + store ----
                for it in range(NT):
                    f_pool = psumO if it % 2 == 0 else psum
                    f_ps = f_pool.tile([P, N], F32,
                                       tag="ops" if it % 2 == 0 else "ps",
                                       name="f5_ps")
                    for (c0, cw) in ((0, 512), (512, 256)):
                        nc.tensor.matmul(
                            f_ps[:, c0:c0 + cw],
                            lhsT=aT[5][:, it * P:(it + 1) * P],
                            rhs=wo_sb[5][:, c0:c0 + cw],
                            start=True, stop=True)
                    o_sb = outp.tile([P, D], F32, tag="osb", name="osb_t")
                    nc.vector.tensor_add(o_sb[:], f_ps[:, 0:D], acc[it][:])
                    nc.sync.dma_start(out[it * P:(it + 1) * P, :], o_sb[:])

    nc.compile()
    return nc


def _host_tables():
    inv_freq = 1.0 / (10000.0 ** (np.arange(0, DH, 2, dtype=np.float32) / DH))
    t = np.arange(N, dtype=np.float32)
    freqs = np.einsum("i,j->ij", t, inv_freq)          # [N, 32]
    emb = np.concatenate([freqs, freqs], axis=-1)      # [N, 64]
    cosT = np.cos(emb).T.astype(np.float32)            # [64, N]
    sinT = np.sin(emb).T.astype(np.float32)            # [64, N]
    # b-term: out rows 0:32 use -sin (pair d+32), rows 32:64 use +sin
    sins = np.concatenate([-sinT[0:32], sinT[32:64]], axis=0)  # [64, N]
    cos2 = np.concatenate([cosT, cosT], axis=0)        # [128, N]
    sins2 = np.concatenate([sins, sins], axis=0)       # [128, N]
    # pre-swap 32-row blocks (0<->32, 64<->96): the device multiplies
    # qb[ob] = qf[ib] * sin_sb[ib], so sin_sb[ib] must hold sins2[ob].
    sinsw2 = np.concatenate(
        [sins2[32:64], sins2[0:32], sins2[96:128], sins2[64:96]], axis=0)
    return np.ascontiguousarray(cos2), np.ascontiguousarray(sinsw2)


def kernel(x, w_qkv, w_out, b_out):
    from concourse.bass_utils import run_bass_kernel_spmd

    if "nc" not in _CACHE:
        _CACHE["nc"] = _build()
    nc = _CACHE["nc"]

    bf = ml_dtypes.bfloat16
    cos2, sins2 = _host_tables()
    cos2 = np.ascontiguousarray(cos2.astype(bf))
    sins2 = np.ascontiguousarray(sins2.astype(bf))
    biasb = np.ascontiguousarray(
        np.broadcast_to(np.asarray(b_out, np.float32)[None, :], (P, D)))

    def _sbufize(w):   # [(k p), e] -> [p, (k e)] exact SBUF layout
        w = np.asarray(w, np.float32).astype(bf)
        k, e = w.shape[0] // P, w.shape[1]
        return np.ascontiguousarray(
            w.reshape(k, P, e).transpose(1, 0, 2).reshape(P, k * e))

    # pair-major qk weights: [p, hp*1536 + k*256 + u*128 + c], m = hp + 6u
    w1 = np.asarray(w_qkv, np.float32)[:, 0:1536].astype(bf)
    wqkp = np.ascontiguousarray(
        w1.reshape(KT, P, 2, 6, P).transpose(1, 3, 0, 2, 4)
        .reshape(P, KT * 1536))
    wv_b = _sbufize(np.asarray(w_qkv, np.float32)[:, 1536:E3])
    wout_b = _sbufize(w_out)

    in_maps = []
    for i in range(N_CORES):
        xi = np.ascontiguousarray(
            np.asarray(x[i], np.float32).astype(bf).T)
        in_maps.append({
            "x": xi, "wqkp": wqkp, "wv_d": wv_b, "wout": wout_b,
            "cos2": cos2, "sins2": sins2, "biasb": biasb,
        })

    res = run_bass_kernel_spmd(
        nc, in_maps, list(range(N_CORES)),
        trace=bool(int(os.environ.get("KERNEL_TRACE", "0"))))
    _CACHE["last_result"] = res
    return np.stack([res.results[i]["out"] for i in range(N_CORES)], axis=0)
